# revision 26
# baseline (speedup 1.0000x reference)
"""Trainium2 Bass kernel for nn_ExchangeBlock (gnn_message_passing).

Data-parallel over edges: each of the 8 cores processes E/8 = 16384 edges,
node features and weights replicated. Per 512-edge tile:
  - one batched indirect-DMA gather of [pos|nodes] rows for src/dst + cell rows
  - radial: tvec, dist (DVE Newton rsqrt), Bessel embedding (range-reduced Sin)
  - PE-transposes to feature-major; fp32r matmuls (N=512) for the
    distance-filter MLP, the symmetrized tensor product, and the mix MLP;
    LayerNorm via ones-matmul cross-partition reductions.
mi_w2 is streamed from HBM per output chunk to fit SBUF; the df_w2 matmul
accumulates in the PSUM banks freed by the tensor-product output.
"""
import os
import sys

sys.path.insert(0, "/opt/trn_rl_repo")

import math
import numpy as np

L0, L1, L2 = 32, 16, 8
NS, NB = 512, 256
CUT = 7.0
N, E, G = 16384, 131072, 16
FEAT = L0 + 3 * L1 + 5 * L2  # 120
ROW = 124  # pos(3) + feat(120) + pad(1)
NCORES = 8
EC = E // NCORES  # edges per core
BLK = 128
ET = 512  # edges per tile (= one PSUM bank of fp32)
NBLK = ET // BLK
FAN = math.sqrt(float(L0 * L0 + L1 * L1 + L2 * L2))
EMBC = math.sqrt(2.0 / CUT)
MAGIC = 0x5F3759DF

_cache = {}


def _build(mode, ntiles, reps=1):
    """Build the Bass program (shared by all cores, SPMD)."""
    import concourse.bacc as bacc
    import concourse.bass as bass
    import concourse.mybir as mybir
    import concourse.tile as tile

    f32 = mybir.dt.float32
    f32r = mybir.dt.float32r
    i32 = mybir.dt.int32
    AF = mybir.ActivationFunctionType
    OP = mybir.AluOpType
    AX = mybir.AxisListType

    nc = bacc.Bacc(None)

    # ---------------- DRAM tensors ----------------
    nodesP = nc.dram_tensor("nodesP", [N, ROW], f32, kind="ExternalInput")
    cellpad = nc.dram_tensor("cellpad", [G, 9], f32, kind="ExternalInput")
    srcidx = nc.dram_tensor("srcidx", [ntiles, BLK, NBLK], i32, kind="ExternalInput")
    dstidx = nc.dram_tensor("dstidx", [ntiles, BLK, NBLK], i32, kind="ExternalInput")
    celidx = nc.dram_tensor("celidx", [ntiles, BLK, NBLK], i32, kind="ExternalInput")
    shiftd = nc.dram_tensor("shiftd", [ntiles, BLK, NBLK, 3], f32, kind="ExternalInput")

    w0f = nc.dram_tensor("w0f", [1024, NS], f32, kind="ExternalInput")
    w1f = nc.dram_tensor("w1f", [256, NS], f32, kind="ExternalInput")
    w2f = nc.dram_tensor("w2f", [64, NS], f32, kind="ExternalInput")
    dfw1 = nc.dram_tensor("dfw1", [256, 1024], f32, kind="ExternalInput")
    dfw2 = nc.dram_tensor("dfw2", [1024, NS], f32, kind="ExternalInput")
    miw1 = nc.dram_tensor("miw1", [NS, 1024], f32, kind="ExternalInput")
    miw2 = nc.dram_tensor("miw2", [1024, 1024], f32, kind="ExternalInput")
    mow = nc.dram_tensor("mow", [1024, 1], f32, kind="ExternalInput")
    bdf1 = nc.dram_tensor("bdf1", [BLK, 8], f32, kind="ExternalInput")
    bdf2 = nc.dram_tensor("bdf2", [BLK, 4], f32, kind="ExternalInput")
    bmi1 = nc.dram_tensor("bmi1", [BLK, 8], f32, kind="ExternalInput")
    bmi2 = nc.dram_tensor("bmi2", [BLK, 8], f32, kind="ExternalInput")
    bmo = nc.dram_tensor("bmo", [1, 1], f32, kind="ExternalInput")

    s0d = nc.dram_tensor("s0d", [ROW, 8 * BLK], f32, kind="ExternalInput")
    t0d = nc.dram_tensor("t0d", [ROW, BLK], f32, kind="ExternalInput")
    s1d = nc.dram_tensor("s1d", [ROW, 6 * BLK], f32, kind="ExternalInput")
    t1d = nc.dram_tensor("t1d", [ROW, 3 * BLK], f32, kind="ExternalInput")
    s2d = nc.dram_tensor("s2d", [ROW, 5 * 64], f32, kind="ExternalInput")
    t2d = nc.dram_tensor("t2d", [ROW, 5 * 64], f32, kind="ExternalInput")
    onesd = nc.dram_tensor("onesd", [BLK, 1], f32, kind="ExternalInput")
    onesr = nc.dram_tensor("onesr", [64, BLK], f32, kind="ExternalInput")
    cnd = nc.dram_tensor("cnd", [BLK, NB], f32, kind="ExternalInput")
    identd = nc.dram_tensor("identd", [BLK, BLK], f32, kind="ExternalInput")

    outd = nc.dram_tensor("out", [ntiles, 1, ET], f32, kind="ExternalOutput")

    TWO_PI = 2.0 * math.pi
    sin_bias = -math.pi if mode == "sim" else 0.0
    dscale_c = -EMBC if mode == "sim" else EMBC

    with tile.TileContext(nc) as tc:
        with (
            tc.tile_pool(name="const", bufs=1) as cp,
            tc.tile_pool(name="wstr", bufs=2) as wsp,
            tc.tile_pool(name="gat", bufs=2) as gp,
            tc.tile_pool(name="rad", bufs=2) as rp,
            tc.tile_pool(name="row", bufs=1) as wp,
            tc.tile_pool(name="emb", bufs=1) as ep,
            tc.tile_pool(name="trx", bufs=1) as xp,
            tc.tile_pool(name="tpa", bufs=1) as tpa,
            tc.tile_pool(name="tpb", bufs=1) as tpb,
            tc.tile_pool(name="mid", bufs=1) as mp,
            tc.tile_pool(name="sml", bufs=2) as sp,
            tc.tile_pool(name="big", bufs=1) as bp,
            tc.tile_pool(name="h1s", bufs=2) as hp,
            tc.tile_pool(name="psw", bufs=4, space="PSUM") as psw,
            tc.tile_pool(name="psmix", bufs=1, space="PSUM") as psm,
        ):
            # ---------------- constants ----------------
            w0_t = cp.tile([BLK, 8, NS], f32r)
            for c in range(8):
                nc.sync.dma_start(w0_t[:, c, :], w0f[c * BLK:(c + 1) * BLK, :].bitcast(f32r))
            w1_t = cp.tile([BLK, 2, NS], f32r)
            for c in range(2):
                nc.sync.dma_start(w1_t[:, c, :], w1f[c * BLK:(c + 1) * BLK, :].bitcast(f32r))
            w2_t = cp.tile([64, NS], f32r)
            nc.sync.dma_start(w2_t[:], w2f[:].bitcast(f32r))
            dfw2_t = cp.tile([BLK, 8, NS], f32r)
            for c in range(8):
                nc.sync.dma_start(dfw2_t[:, c, :], dfw2[c * BLK:(c + 1) * BLK, :].bitcast(f32r))
            miw1_t = cp.tile([BLK, 4, 1024], f32r)
            for c in range(4):
                nc.sync.dma_start(miw1_t[:, c, :], miw1[c * BLK:(c + 1) * BLK, :].bitcast(f32r))
            miw2_t = cp.tile([BLK, 8, 1024], f32r)
            for c in range(8):
                nc.sync.dma_start(miw2_t[:, c, :], miw2[c * BLK:(c + 1) * BLK, :].bitcast(f32r))
            mow_t = cp.tile([BLK, 8], f32r)
            nc.sync.dma_start(mow_t[:], mow[:].rearrange("(c p) one -> p (c one)", p=BLK).bitcast(f32r))
            bdf1_t = cp.tile([BLK, 8], f32)
            nc.sync.dma_start(bdf1_t[:], bdf1[:])
            bdf2_t = cp.tile([BLK, 4], f32)
            nc.sync.dma_start(bdf2_t[:], bdf2[:])
            bmi1_t = cp.tile([BLK, 8], f32)
            nc.sync.dma_start(bmi1_t[:], bmi1[:])
            bmi2_t = cp.tile([BLK, 8], f32)
            nc.sync.dma_start(bmi2_t[:], bmi2[:])
            bmo_t = cp.tile([1, 1], f32)
            nc.sync.dma_start(bmo_t[:], bmo[:])
            s0_t = cp.tile([ROW, 8 * BLK], f32r)
            nc.sync.dma_start(s0_t[:], s0d[:].bitcast(f32r))
            t0_t = cp.tile([ROW, BLK], f32r)
            nc.sync.dma_start(t0_t[:], t0d[:].bitcast(f32r))
            s1_t = cp.tile([ROW, 6 * BLK], f32r)
            nc.sync.dma_start(s1_t[:], s1d[:].bitcast(f32r))
            t1_t = cp.tile([ROW, 3 * BLK], f32r)
            nc.sync.dma_start(t1_t[:], t1d[:].bitcast(f32r))
            s2_t = cp.tile([ROW, 5 * 64], f32r)
            nc.sync.dma_start(s2_t[:], s2d[:].bitcast(f32r))
            t2_t = cp.tile([ROW, 5 * 64], f32r)
            nc.sync.dma_start(t2_t[:], t2d[:].bitcast(f32r))
            ones_t = cp.tile([BLK, 1], f32r)
            nc.sync.dma_start(ones_t[:], onesd[:].bitcast(f32r))
            onesr_t = cp.tile([64, BLK], f32r)
            nc.sync.dma_start(onesr_t[:], onesr[:].bitcast(f32r))
            cn_t = cp.tile([BLK, NB], f32)
            nc.sync.dma_start(cn_t[:], cnd[:])
            id_t = cp.tile([BLK, BLK], f32)
            nc.sync.dma_start(id_t[:], identd[:])
            magic_t = cp.tile([BLK, NS], i32)
            nc.gpsimd.memset(magic_t[:], MAGIC)
            sinb_t = cp.tile([BLK, 1], f32)
            nc.gpsimd.memset(sinb_t[:], sin_bias)

            def silu_to(dst, ps, bias_ap):
                if mode == "sim":
                    sg = sp.tile([BLK, ET], f32, tag="sg")
                    nc.scalar.activation(sg[:], ps, AF.Sigmoid, bias=bias_ap, scale=1.0)
                    pre = sp.tile([BLK, ET], f32, tag="pre")
                    nc.vector.tensor_scalar(out=pre[:], in0=ps, scalar1=bias_ap,
                                            scalar2=None, op0=OP.add)
                    nc.vector.tensor_tensor(out=dst, in0=sg[:], in1=pre[:], op=OP.mult)
                else:
                    nc.scalar.activation(dst, ps, AF.Silu, bias=bias_ap, scale=1.0)

            for t in [tt for _ in range(reps) for tt in range(ntiles)]:
                # ============ gathers ============
                sidx = gp.tile([BLK, NBLK], i32, tag="sidx")
                didx = gp.tile([BLK, NBLK], i32, tag="didx")
                cidx = gp.tile([BLK, NBLK], i32, tag="cidx")
                nc.sync.dma_start(sidx[:], srcidx[t])
                nc.sync.dma_start(didx[:], dstidx[t])
                nc.sync.dma_start(cidx[:], celidx[t])
                shf = gp.tile([BLK, NBLK, 3], f32, tag="shf")
                nc.sync.dma_start(shf[:], shiftd[t])
                gs = gp.tile([BLK, NBLK, ROW], f32, tag="gs")
                gd = gp.tile([BLK, NBLK, ROW], f32, tag="gd")
                gc = gp.tile([BLK, NBLK, 9], f32, tag="gc")
                for b in range(NBLK):
                    nc.gpsimd.indirect_dma_start(
                        out=gs[:, b, :], out_offset=None, in_=nodesP[:],
                        in_offset=bass.IndirectOffsetOnAxis(ap=sidx[:, b:b + 1], axis=0))
                    nc.gpsimd.indirect_dma_start(
                        out=gd[:, b, :], out_offset=None, in_=nodesP[:],
                        in_offset=bass.IndirectOffsetOnAxis(ap=didx[:, b:b + 1], axis=0))
                    nc.gpsimd.indirect_dma_start(
                        out=gc[:, b, :], out_offset=None, in_=cellpad[:],
                        in_offset=bass.IndirectOffsetOnAxis(ap=cidx[:, b:b + 1], axis=0))

                # ============ radial ============
                prod = rp.tile([BLK, NBLK, 3, 3], f32, tag="prod")
                nc.vector.tensor_tensor(
                    out=prod[:],
                    in0=gc[:].rearrange("p b (i j) -> p b j i", i=3, j=3),
                    in1=shf[:].unsqueeze(2).to_broadcast([BLK, NBLK, 3, 3]),
                    op=OP.mult)
                tvec = rp.tile([BLK, NBLK, 3], f32, tag="tvec")
                nc.vector.tensor_reduce(out=tvec[:], in_=prod[:], axis=AX.X, op=OP.add)
                rv = rp.tile([BLK, NBLK, 3], f32, tag="rv")
                nc.vector.tensor_tensor(out=rv[:], in0=gd[:, :, 0:3], in1=gs[:, :, 0:3], op=OP.subtract)
                nc.vector.tensor_tensor(out=rv[:], in0=rv[:], in1=tvec[:], op=OP.add)
                sq = rp.tile([BLK, NBLK, 3], f32, tag="sq")
                nc.vector.tensor_tensor(out=sq[:], in0=rv[:], in1=rv[:], op=OP.mult)
                d2 = rp.tile([BLK, NBLK], f32, tag="d2")
                nc.vector.tensor_reduce(out=d2[:], in_=sq[:], axis=AX.X, op=OP.add)
                nc.vector.tensor_scalar(out=d2[:], in0=d2[:], scalar1=1e-24, scalar2=None, op0=OP.max)
                # Newton rsqrt
                sh = rp.tile([BLK, NBLK], i32, tag="sh")
                nc.vector.tensor_scalar(out=sh[:], in0=d2[:].bitcast(i32), scalar1=1,
                                        scalar2=None, op0=OP.arith_shift_right)
                yi = rp.tile([BLK, NBLK], i32, tag="yi")
                nc.vector.tensor_tensor(out=yi[:], in0=magic_t[:, 0:NBLK], in1=sh[:], op=OP.subtract)
                y = yi[:].bitcast(f32)
                d2h = rp.tile([BLK, NBLK], f32, tag="d2h")
                nc.vector.tensor_scalar(out=d2h[:], in0=d2[:], scalar1=0.5, scalar2=None, op0=OP.mult)
                tmp = rp.tile([BLK, NBLK], f32, tag="tmp")
                for _ in range(3):
                    nc.vector.tensor_tensor(out=tmp[:], in0=y, in1=y, op=OP.mult)
                    nc.vector.tensor_tensor(out=tmp[:], in0=tmp[:], in1=d2h[:], op=OP.mult)
                    nc.vector.tensor_scalar(out=tmp[:], in0=tmp[:], scalar1=-1.0, scalar2=1.5,
                                            op0=OP.mult, op1=OP.add)
                    nc.vector.tensor_tensor(out=yi[:].bitcast(f32), in0=y, in1=tmp[:], op=OP.mult)
                dist = rp.tile([BLK, NBLK], f32, tag="dist")
                nc.vector.tensor_tensor(out=dist[:], in0=d2[:], in1=y, op=OP.mult)
                nc.vector.tensor_scalar(out=dist[:], in0=dist[:], scalar1=1e-6, scalar2=None, op0=OP.add)
                # r = 1/(dist+1e-6), one NR step from seed y
                nc.vector.tensor_tensor(out=tmp[:], in0=dist[:], in1=y, op=OP.mult)
                nc.vector.tensor_scalar(out=tmp[:], in0=tmp[:], scalar1=-1.0, scalar2=2.0,
                                        op0=OP.mult, op1=OP.add)
                r_ = rp.tile([BLK, NBLK], f32, tag="r_")
                nc.vector.tensor_tensor(out=r_[:], in0=y, in1=tmp[:], op=OP.mult)
                dsc = rp.tile([BLK, NBLK], f32, tag="dsc")
                nc.vector.tensor_scalar(out=dsc[:], in0=dist[:], scalar1=dscale_c, scalar2=None, op0=OP.mult)

                # ============ embedding (edge-major) ============
                u = ep.tile([BLK, NBLK, NB], f32, tag="u")
                nc.vector.tensor_tensor(
                    out=u[:], in0=r_[:].unsqueeze(2).to_broadcast([BLK, NBLK, NB]),
                    in1=cn_t[:].unsqueeze(1).to_broadcast([BLK, NBLK, NB]), op=OP.mult)
                icv = ep.tile([BLK, NBLK, NB], i32, tag="icv")
                nc.vector.tensor_copy(icv[:], u[:])
                nc.vector.tensor_tensor(out=u[:], in0=u[:], in1=icv[:], op=OP.subtract)
                sinv = ep.tile([BLK, NBLK, NB], f32, tag="icv")
                nc.scalar.activation(sinv[:], u[:], AF.Sin, bias=sinb_t[:, 0:1], scale=TWO_PI)
                nc.vector.tensor_tensor(
                    out=sinv[:], in0=sinv[:],
                    in1=dsc[:].unsqueeze(2).to_broadcast([BLK, NBLK, NB]), op=OP.mult)

                # ============ transposes ============
                embT = xp.tile([BLK, 2, ET], f32r, tag="embT")
                for b in range(NBLK):
                    for m in range(2):
                        pt = psw.tile([BLK, NS], f32, space="PSUM", tag="w")
                        nc.tensor.transpose(pt[:BLK, :BLK], sinv[:, b, m * BLK:(m + 1) * BLK], id_t[:])
                        nc.scalar.copy(embT[:, m, b * BLK:(b + 1) * BLK], pt[:BLK, :BLK])
                xsT = xp.tile([ROW, ET], f32r, tag="xsT")
                xdT = xp.tile([ROW, ET], f32r, tag="xdT")
                for b in range(NBLK):
                    pt = psw.tile([BLK, NS], f32, space="PSUM", tag="w")
                    nc.tensor.transpose(pt[:ROW, :BLK], gs[:, b, :], id_t[:])
                    nc.scalar.copy(xsT[:, b * BLK:(b + 1) * BLK], pt[:ROW, :BLK])
                    pt = psw.tile([BLK, NS], f32, space="PSUM", tag="w")
                    nc.tensor.transpose(pt[:ROW, :BLK], gd[:, b, :], id_t[:])
                    nc.scalar.copy(xdT[:, b * BLK:(b + 1) * BLK], pt[:ROW, :BLK])
                xsr = xsT[:].bitcast(f32r)
                xdr = xdT[:].bitcast(f32r)

                # ============ tensor product -> mixed psum [128, 4, 512] ============
                mix_ps = psm.tile([BLK, 4, NS], f32, space="PSUM", tag="mix")
                psB0 = psw.tile([BLK, NS], f32, space="PSUM", tag="w")
                nc.tensor.matmul(psB0[:], t0_t[:], xdr, start=True, stop=True)
                xd0t = tpb.tile([BLK, ET], f32r, tag="xd0t")
                nc.scalar.copy(xd0t[:], psB0[:])
                for c in range(8):
                    psA = psw.tile([BLK, NS], f32, space="PSUM", tag="w")
                    nc.tensor.matmul(psA[:], s0_t[:, c * BLK:(c + 1) * BLK], xsr,
                                     start=True, stop=True)
                    e0c = tpa.tile([BLK, ET], f32r, tag="e0c")
                    nc.vector.tensor_tensor(out=e0c[:], in0=psA[:], in1=xd0t[:], op=OP.mult)
                    for m in range(4):
                        nc.tensor.matmul(mix_ps[:, m, :], w0_t[:, c, m * BLK:(m + 1) * BLK], e0c[:],
                                         start=(c == 0), stop=False)
                # e1 (i outer so the B broadcast is copied to SBUF once per i)
                e1acc = tpb.tile([BLK, 2, ET], f32r, tag="e1c")
                for i in range(3):
                    psB = psw.tile([BLK, NS], f32, space="PSUM", tag="w")
                    nc.tensor.matmul(psB[:], t1_t[:, i * BLK:(i + 1) * BLK], xdr,
                                     start=True, stop=True)
                    bsb = tpb.tile([BLK, ET], f32, tag="bsb")
                    nc.scalar.copy(bsb[:], psB[:])
                    for c in range(2):
                        psA = psw.tile([BLK, NS], f32, space="PSUM", tag="w")
                        nc.tensor.matmul(psA[:], s1_t[:, (c * 3 + i) * BLK:(c * 3 + i + 1) * BLK],
                                         xsr, start=True, stop=True)
                        if i == 0:
                            nc.vector.tensor_tensor(out=e1acc[:, c, :], in0=psA[:], in1=bsb[:], op=OP.mult)
                        else:
                            tmq = tpb.tile([BLK, ET], f32, tag="tmq")
                            nc.vector.tensor_tensor(out=tmq[:], in0=psA[:], in1=bsb[:], op=OP.mult)
                            nc.vector.tensor_tensor(out=e1acc[:, c, :], in0=e1acc[:, c, :], in1=tmq[:], op=OP.add)
                for c in range(2):
                    for m in range(4):
                        nc.tensor.matmul(mix_ps[:, m, :], w1_t[:, c, m * BLK:(m + 1) * BLK], e1acc[:, c, :],
                                         start=False, stop=False)
                # e2
                e2t = tpb.tile([64, ET], f32r, tag="e2t")
                for i in range(5):
                    psB = psw.tile([BLK, NS], f32, space="PSUM", tag="w")
                    nc.tensor.matmul(psB[:64, :], t2_t[:, i * 64:(i + 1) * 64], xdr,
                                     start=True, stop=True)
                    bsb2 = tpb.tile([64, ET], f32, tag="bsb")
                    nc.scalar.copy(bsb2[:], psB[:64, :])
                    psA = psw.tile([BLK, NS], f32, space="PSUM", tag="w")
                    nc.tensor.matmul(psA[:64, :], s2_t[:, i * 64:(i + 1) * 64], xsr,
                                     start=True, stop=True)
                    if i == 0:
                        nc.vector.tensor_tensor(out=e2t[:], in0=psA[:64, :], in1=bsb2[:], op=OP.mult)
                    else:
                        tmq2 = tpb.tile([64, ET], f32, tag="tmq")
                        nc.vector.tensor_tensor(out=tmq2[:], in0=psA[:64, :], in1=bsb2[:], op=OP.mult)
                        nc.vector.tensor_tensor(out=e2t[:], in0=e2t[:], in1=tmq2[:], op=OP.add)
                for m in range(4):
                    nc.tensor.matmul(mix_ps[:, m, :], w2_t[:, m * BLK:(m + 1) * BLK], e2t[:],
                                     start=False, stop=True)

                # copy mixed to sbuf (PE cannot read PSUM); frees mix banks for df2
                mix_sb = mp.tile([BLK, 4, NS], f32r, tag="mix_sb")
                for m in range(4):
                    nc.scalar.copy(mix_sb[:, m, :], mix_ps[:, m, :])

                # ============ LayerNorm stats ============
                mu_ps = psw.tile([BLK, NS], f32, space="PSUM", tag="w")
                for m in range(4):
                    nc.tensor.matmul(mu_ps[0:1, :], ones_t[:], mix_sb[:, m, :],
                                     start=(m == 0), stop=(m == 3))
                s2_ps = psw.tile([BLK, NS], f32, space="PSUM", tag="w")
                sqf = mp.tile([BLK, 4, NS], f32r, tag="sqf")
                nc.vector.tensor_tensor(out=sqf[:], in0=mix_sb[:], in1=mix_sb[:], op=OP.mult)
                for m in range(4):
                    nc.tensor.matmul(s2_ps[0:1, :], ones_t[:], sqf[:, m, :],
                                     start=(m == 0), stop=(m == 3))
                # broadcast the stat rows first, then full-width stats math
                rmu = wp.tile([1, NS], f32r, tag="rr")
                nc.scalar.copy(rmu[:], mu_ps[0:1, :])
                mub_ps = psw.tile([BLK, NS], f32, space="PSUM", tag="w")
                nc.tensor.matmul(mub_ps[:], onesr_t[0:1, :], rmu[:], start=True, stop=True)
                rs2 = wp.tile([1, NS], f32r, tag="rr")
                nc.scalar.copy(rs2[:], s2_ps[0:1, :])
                s2b_ps = psw.tile([BLK, NS], f32, space="PSUM", tag="w")
                nc.tensor.matmul(s2b_ps[:], onesr_t[0:1, :], rs2[:], start=True, stop=True)
                stats4 = mp.tile([BLK, 4, NS], f32, tag="sqf")
                vb = stats4[:, 0, :]
                mub = stats4[:, 1, :]
                tb = stats4[:, 2, :]
                yb = stats4[:, 3, :]
                nc.vector.tensor_scalar(out=vb, in0=s2b_ps[:], scalar1=1.0 / NS,
                                        scalar2=1e-5, op0=OP.mult, op1=OP.add)
                nc.vector.tensor_scalar(out=mub, in0=mub_ps[:], scalar1=1.0 / NS,
                                        scalar2=None, op0=OP.mult)
                nc.vector.tensor_tensor(out=tb, in0=mub, in1=mub, op=OP.mult)
                nc.vector.tensor_tensor(out=vb, in0=vb, in1=tb, op=OP.subtract)
                nc.vector.tensor_scalar(out=yb.bitcast(i32), in0=vb.bitcast(i32), scalar1=1,
                                        scalar2=None, op0=OP.arith_shift_right)
                nc.vector.tensor_tensor(out=yb.bitcast(i32), in0=magic_t[:],
                                        in1=yb.bitcast(i32), op=OP.subtract)
                for _ in range(2):
                    nc.vector.tensor_tensor(out=tb, in0=yb, in1=yb, op=OP.mult)
                    nc.vector.tensor_tensor(out=tb, in0=tb, in1=vb, op=OP.mult)
                    nc.vector.tensor_scalar(out=tb, in0=tb, scalar1=-0.5, scalar2=1.5,
                                            op0=OP.mult, op1=OP.add)
                    nc.vector.tensor_tensor(out=yb, in0=yb, in1=tb, op=OP.mult)
                nc.vector.tensor_tensor(out=mub, in0=mub, in1=yb, op=OP.mult)

                # ============ distance-filter MLP (h1 chunk-streamed) ============
                # df2 accumulates in the PSUM banks released by mix_ps
                df_ps = psm.tile([BLK, 4, NS], f32, space="PSUM", tag="mix")
                for kc in range(8):
                    df1s = wsp.tile([BLK, 2, BLK], f32r, tag="df1s")
                    nc.sync.dma_start(
                        df1s[:],
                        dfw1[:, kc * BLK:(kc + 1) * BLK].rearrange(
                            "(c p) m -> p c m", p=BLK).bitcast(f32r))
                    ph = psw.tile([BLK, NS], f32, space="PSUM", tag="w")
                    for k2 in range(2):
                        nc.tensor.matmul(ph[:], df1s[:, k2, :],
                                         embT[:, k2, :], start=(k2 == 0), stop=(k2 == 1))
                    h1c = hp.tile([BLK, ET], f32r, tag="h1c")
                    silu_to(h1c[:], ph[:], bdf1_t[:, kc:kc + 1])
                    for m in range(4):
                        nc.tensor.matmul(df_ps[:, m, :], dfw2_t[:, kc, m * BLK:(m + 1) * BLK],
                                         h1c[:], start=(kc == 0), stop=(kc == 7))
                dff = mp.tile([BLK, 4, NS], f32r, tag="dff")
                for m in range(4):
                    nc.scalar.activation(dff[:, m, :], df_ps[:, m, :], AF.Identity,
                                         bias=bdf2_t[:, m:m + 1], scale=1.0)

                # ============ reg = (mixed*rstd - mu*rstd) * df ============
                for m in range(4):
                    xg = sp.tile([BLK, ET], f32, tag="xg")
                    nc.vector.tensor_tensor(out=xg[:], in0=mix_sb[:, m, :], in1=yb, op=OP.mult)
                    nc.vector.tensor_tensor(out=xg[:], in0=xg[:], in1=mub, op=OP.subtract)
                    nc.vector.tensor_tensor(out=dff[:, m, :], in0=dff[:, m, :], in1=xg[:], op=OP.mult)
                reg = dff

                # ============ mix MLP ============
                h = bp.tile([BLK, 8, ET], f32r, tag="h")
                for m in range(8):
                    ph = psw.tile([BLK, NS], f32, space="PSUM", tag="w")
                    for kc in range(4):
                        nc.tensor.matmul(ph[:], miw1_t[:, kc, m * BLK:(m + 1) * BLK],
                                         reg[:, kc, :], start=(kc == 0), stop=(kc == 3))
                    silu_to(h[:, m, :], ph[:], bmi1_t[:, m:m + 1])
                po = psw.tile([BLK, NS], f32, space="PSUM", tag="w")
                for m in range(8):
                    ph = psw.tile([BLK, NS], f32, space="PSUM", tag="w")
                    for kc in range(8):
                        nc.tensor.matmul(ph[:], miw2_t[:, kc, m * BLK:(m + 1) * BLK], h[:, kc, :],
                                         start=(kc == 0), stop=(kc == 7))
                    h2m = sp.tile([BLK, ET], f32r, tag="h2m")
                    silu_to(h2m[:], ph[:], bmi2_t[:, m:m + 1])
                    nc.tensor.matmul(po[0:1, :], mow_t[:, m:m + 1], h2m[:],
                                     start=(m == 0), stop=(m == 7))
                ot = wp.tile([1, ET], f32, tag="ot")
                nc.scalar.activation(ot[:], po[0:1, :], AF.Identity, bias=bmo_t[:, 0:1], scale=1.0)
                nc.sync.dma_start(outd[t], ot[:])

    nc.finalize()
    return nc


def _host_prep(inputs):
    """Shared (replicated) host-side tensors."""
    f = np.float32
    nodes = np.asarray(inputs["nodes"], f)
    pos = np.asarray(inputs["pos"], f)
    cell = np.asarray(inputs["cell"], f)
    W0 = np.asarray(inputs["W0"], f)
    W1 = np.asarray(inputs["W1"], f)
    W2 = np.asarray(inputs["W2"], f)
    ln_g = np.asarray(inputs["ln_g"], f)

    nodesP = np.concatenate([pos, nodes, np.zeros((N, 1), f)], axis=1)
    cellpad = np.ascontiguousarray(cell.reshape(G, 9))

    sym = lambda W: 0.5 * (W + W.transpose(1, 0, 2))
    w0f = np.ascontiguousarray((sym(W0) / FAN).reshape(L0 * L0, NS))
    w1f = np.ascontiguousarray((sym(W1) / (FAN * math.sqrt(3.0))).reshape(L1 * L1, NS))
    w2f = np.ascontiguousarray((sym(W2) / (FAN * math.sqrt(5.0))).reshape(L2 * L2, NS))
    miw1 = np.ascontiguousarray(ln_g[:, None] * np.asarray(inputs["mi_w1"], f))

    def colbias(b, nch):
        b = np.asarray(b, f).reshape(nch, BLK)
        return np.ascontiguousarray(b.T)

    O0 = 3
    O1 = 3 + L0
    O2 = 3 + L0 + 3 * L1
    s0 = np.zeros((ROW, 8 * BLK), f)
    for c in range(8):
        for p in range(BLK):
            s0[O0 + c * 4 + p // 32, c * BLK + p] = 1.0
    t0 = np.zeros((ROW, BLK), f)
    for p in range(BLK):
        t0[O0 + p % 32, p] = 1.0
    s1 = np.zeros((ROW, 6 * BLK), f)
    for c in range(2):
        for i in range(3):
            for p in range(BLK):
                u = c * 8 + p // 16
                s1[O1 + u * 3 + i, (c * 3 + i) * BLK + p] = 1.0
    t1 = np.zeros((ROW, 3 * BLK), f)
    for i in range(3):
        for p in range(BLK):
            t1[O1 + (p % 16) * 3 + i, i * BLK + p] = 1.0
    s2 = np.zeros((ROW, 5 * 64), f)
    t2 = np.zeros((ROW, 5 * 64), f)
    for i in range(5):
        for p in range(64):
            s2[O2 + (p // 8) * 5 + i, i * 64 + p] = 1.0
            t2[O2 + (p % 8) * 5 + i, i * 64 + p] = 1.0
    cn = np.broadcast_to((np.arange(1, NB + 1, dtype=f) / (2.0 * CUT))[None, :], (BLK, NB)).copy()

    def _onesr():
        o = np.zeros((64, BLK), f)
        o[0, :] = 1.0
        o[32, :] = 1.0
        return o

    return dict(
        nodesP=nodesP, cellpad=cellpad,
        w0f=w0f, w1f=w1f, w2f=w2f,
        dfw1=np.asarray(inputs["df_w1"], f), dfw2=np.asarray(inputs["df_w2"], f),
        miw1=miw1, miw2=np.asarray(inputs["mi_w2"], f),
        mow=np.asarray(inputs["mo_w"], f),
        bdf1=colbias(inputs["df_b1"], 8), bdf2=colbias(inputs["df_b2"], 4),
        bmi1=colbias(inputs["mi_b1"], 8), bmi2=colbias(inputs["mi_b2"], 8),
        bmo=np.asarray(inputs["mo_b"], f).reshape(1, 1),
        s0d=s0, t0d=t0, s1d=s1, t1d=t1, s2d=s2, t2d=t2,
        onesd=np.ones((BLK, 1), f), onesr=_onesr(), cnd=cn,
        identd=np.eye(BLK, dtype=f),
    )


def _edge_prep(inputs, core, ntiles):
    """Per-core edge tensors."""
    f = np.float32
    ec = ntiles * ET
    lo = core * EC
    ei = np.asarray(inputs["edge_index"])
    src = ei[0, lo:lo + ec].astype(np.int32)
    dst = ei[1, lo:lo + ec].astype(np.int32)
    bv = np.asarray(inputs["batch_vec"]).astype(np.int32)
    cel = bv[src]
    shift = np.asarray(inputs["edge_shift"], f)[lo:lo + ec]

    def tile_idx(x):
        return np.ascontiguousarray(x.reshape(ntiles, NBLK, BLK).transpose(0, 2, 1))

    return dict(
        srcidx=tile_idx(src), dstidx=tile_idx(dst), celidx=tile_idx(cel),
        shiftd=np.ascontiguousarray(
            shift.reshape(ntiles, NBLK, BLK, 3).transpose(0, 2, 1, 3)),
    )


def _run(inputs, mode, ntiles, ncores):
    key = (mode, ntiles, 1)
    if key not in _cache:
        _cache[key] = _build(mode, ntiles)
    nc = _cache[key]
    shared = _host_prep(inputs)
    in_maps = []
    for c in range(ncores):
        m = dict(shared)
        m.update(_edge_prep(inputs, c, ntiles))
        in_maps.append(m)

    if mode == "sim":
        from concourse.bass_interp import CoreSim
        outs = []
        for c in range(ncores):
            sim = CoreSim(nc)
            for k, v in in_maps[c].items():
                sim.tensor(k)[:] = v
            sim.simulate()
            outs.append(np.array(sim.tensor("out")).reshape(-1))
        return np.concatenate(outs).reshape(-1, 1)

    from concourse.bass_utils import run_bass_kernel_spmd
    trace = os.environ.get("EXB_TRACE", "0") == "1"
    res = run_bass_kernel_spmd(nc, in_maps, list(range(ncores)), trace=trace)
    out = np.concatenate([res.results[c]["out"].reshape(-1) for c in range(ncores)])
    if trace:
        _run.last_exec_time_ns = res.exec_time_ns
    return out.reshape(-1, 1)


def kernel(**inputs) -> np.ndarray:
    return _run(inputs, os.environ.get("EXB_MODE", "hw"), EC // ET, NCORES).astype(np.float32)
